# revision 3
# baseline (speedup 1.0000x reference)
"""Multi-head attention (B=2, S=2048, E=1024, H=16, D=64) on 8 Trainium2 cores.

Sharding: data-parallel over batch (2 groups of 4 cores), tensor-parallel over
heads within each group (4 heads per core, Megatron-style column-split qkv);
out_proj sharded over its output columns (each core owns 256 E-columns, the
w_out column slice arrives pre-sliced from the host so the SPMD program needs
no core-id logic).

Per-core pipeline (all matmuls float32r = full PE rate, ~1e-4 rounding):
  x^T (PE transpose, exact fp32) -> q^T/k^T (lhsT=W, rhs=x^T) and v natural
  (lhsT=x^T, rhs=Wv) -> S^T = k q^T per head (row-packed K=64 head pairs)
  -> exp on ScalarE (scale=1/8 folded; no max subtraction: logits ~N(0,1))
  -> PV^T and ones-row denominators (col-packed head pairs, PSUM-accumulated
  over the 16 S_k chunks) -> reciprocal + multiply normalization
  -> AllGather(out^T) over the 4-core group -> out_proj slice.
"""

import numpy as np
from contextlib import ExitStack

import concourse.tile as tile
from concourse import bacc, mybir
from concourse.bass_utils import run_bass_kernel_spmd
from concourse.masks import make_identity

B, S, E, H, D = 2, 2048, 1024, 16, 64
N_CORES = 8
HPC = 4            # heads per core
HD = HPC * D       # 256
GROUPS = [[0, 1, 2, 3], [4, 5, 6, 7]]

F32 = mybir.dt.float32
F32R = mybir.dt.float32r
BF16 = mybir.dt.bfloat16

_cached = None


def build():
    nc = bacc.Bacc("TRN2", target_bir_lowering=False, debug=False,
                   num_devices=N_CORES)

    x_d = nc.dram_tensor("x", [S, E], F32, kind="ExternalInput").ap()
    wq_d = nc.dram_tensor("wq", [E, HD], F32R, kind="ExternalInput").ap()
    wk_d = nc.dram_tensor("wk", [E, HD], F32R, kind="ExternalInput").ap()
    wv_d = nc.dram_tensor("wv", [E, HD], F32R, kind="ExternalInput").ap()
    wo_d = nc.dram_tensor("wo", [E, HD], F32R, kind="ExternalInput").ap()
    y_d = nc.dram_tensor("y", [S, HD], F32, kind="ExternalOutput").ap()
    cc_in = nc.dram_tensor("cc_in", [2 * 128, S], F32R).ap()
    cc_out = nc.dram_tensor("cc_out", [8 * 128, S], F32R).ap()

    with tile.TileContext(nc) as tc, ExitStack() as ctx:
        glob = ctx.enter_context(tc.tile_pool(name="glob", bufs=1))
        qT_t = glob.tile([128, 2, S], F32R, tag="qT")     # q^T by head pair
        kT_t = glob.tile([128, 2, S], F32R, tag="kT")
        v_t = glob.tile([128, 16, HD], BF16, tag="v")     # v natural
        wo_t = glob.tile([128, 8, HD], F32R, tag="wo")
        outT_t = glob.tile([128, 2, S], F32R, tag="outT")
        ident = glob.tile([128, 128], F32, tag="ident")
        ones_f = glob.tile([128, D], F32, tag="ones_f")
        ones_t = glob.tile([128, D], BF16, tag="ones")

        make_identity(nc, ident[:])
        nc.gpsimd.memset(ones_f[:], 1.0)
        nc.vector.tensor_copy(ones_t[:], ones_f[:])
        nc.sync.dma_start(wo_t[:], wo_d.rearrange("(c p) n -> p c n", p=128))

        # ---- Phase A: x^T via PE transpose; Phase B: projections ----
        with ExitStack() as ab:
            abp = ab.enter_context(tc.tile_pool(name="abp", bufs=1))
            xload = ab.enter_context(tc.tile_pool(name="xload", bufs=3))
            tpsum = ab.enter_context(tc.tile_pool(name="tpsum", bufs=2,
                                                  space="PSUM"))
            ppsum = ab.enter_context(tc.tile_pool(name="ppsum", bufs=2,
                                                  space="PSUM"))
            xT_t = abp.tile([128, 8, S], F32R, tag="xT")
            wq_t = abp.tile([128, 8, HD], F32R, tag="wq")
            wk_t = abp.tile([128, 8, HD], F32R, tag="wk")
            wv_t = abp.tile([128, 8, HD], F32R, tag="wv")
            nc.sync.dma_start(wq_t[:], wq_d.rearrange("(c p) n -> p c n", p=128))
            nc.sync.dma_start(wk_t[:], wk_d.rearrange("(c p) n -> p c n", p=128))
            nc.sync.dma_start(wv_t[:], wv_d.rearrange("(c p) n -> p c n", p=128))

            for sc in range(16):
                xt = xload.tile([128, E], F32, tag="x")
                nc.sync.dma_start(xt[:], x_d[sc * 128:(sc + 1) * 128, :])
                for eq in range(2):
                    tp = tpsum.tile([128, 512], F32, tag="tp")
                    for i in range(4):
                        ec = eq * 4 + i
                        nc.tensor.transpose(
                            tp[:, i * 128:(i + 1) * 128],
                            xt[:, ec * 128:(ec + 1) * 128], ident[:])
                    nc.vector.tensor_copy(
                        xT_t[:, eq * 4:(eq + 1) * 4,
                             sc * 128:(sc + 1) * 128],
                        tp[:].rearrange("p (c n) -> p c n", c=4))

            for w_t, dst in ((wq_t, qT_t), (wk_t, kT_t)):
                for mc in range(2):
                    for sq in range(4):
                        pp = ppsum.tile([128, 512], F32, tag="pp")
                        for ec in range(8):
                            nc.tensor.matmul(
                                pp[:],
                                w_t[:, ec, mc * 128:(mc + 1) * 128],
                                xT_t[:, ec, sq * 512:(sq + 1) * 512],
                                start=(ec == 0), stop=(ec == 7))
                        nc.scalar.copy(
                            dst[:, mc, sq * 512:(sq + 1) * 512], pp[:])
            for sc in range(16):
                pp = ppsum.tile([128, 512], F32, tag="pp")
                for ec in range(8):
                    nc.tensor.matmul(
                        pp[:, 0:HD],
                        xT_t[:, ec, sc * 128:(sc + 1) * 128],
                        wv_t[:, ec, :],
                        start=(ec == 0), stop=(ec == 7))
                nc.scalar.copy(v_t[:, sc, :], pp[:, 0:HD])

        # ---- Phase C: attention in transposed space ----
        with ExitStack() as cph:
            stp = cph.enter_context(tc.tile_pool(name="stp", bufs=2,
                                                 space="PSUM"))
            pvp = cph.enter_context(tc.tile_pool(name="pvp", bufs=2,
                                                 space="PSUM"))
            dnp = cph.enter_context(tc.tile_pool(name="dnp", bufs=2,
                                                 space="PSUM"))
            expp = cph.enter_context(tc.tile_pool(name="expp", bufs=3))
            recp = cph.enter_context(tc.tile_pool(name="recp", bufs=2))

            for qc in range(4):
                for hp in range(2):
                    pv = pvp.tile([128, 512], F32, tag="pv")
                    dn = dnp.tile([128, 512], F32, tag="dn")
                    for kc in range(16):
                        st = stp.tile([128, 1024], F32, tag="st")
                        for par in range(2):   # row-packed K=64 pair
                            lo, hi = par * 64, (par + 1) * 64
                            nc.tensor.matmul(
                                st[:, par * 512:(par + 1) * 512],
                                kT_t[lo:hi, hp, kc * 128:(kc + 1) * 128],
                                qT_t[lo:hi, hp, qc * 512:(qc + 1) * 512],
                                start=True, stop=True)
                        ex = expp.tile([128, 1024], BF16, tag="ex")
                        nc.scalar.activation(
                            ex[:], st[:],
                            mybir.ActivationFunctionType.Exp, scale=0.125)
                        for par in range(2):   # col-packed M=64 pairs
                            h = 2 * hp + par
                            lo, hi = par * 64, (par + 1) * 64
                            exs = ex[:, par * 512:(par + 1) * 512]
                            nc.tensor.matmul(
                                pv[lo:hi, :],
                                v_t[:, kc, h * 64:(h + 1) * 64], exs,
                                start=(kc == 0), stop=(kc == 15))
                            nc.tensor.matmul(
                                dn[lo:hi, :], ones_t[:], exs,
                                start=(kc == 0), stop=(kc == 15))
                    rc = recp.tile([128, 512], F32, tag="rc")
                    nc.vector.reciprocal(rc[:], dn[:])
                    nc.vector.tensor_mul(
                        outT_t[:, hp, qc * 512:(qc + 1) * 512], pv[:], rc[:])

        # ---- gather + out_proj (sharded over E columns via host wo slice) ----
        for hp in range(2):
            nc.sync.dma_start(cc_in[hp * 128:(hp + 1) * 128, :],
                              outT_t[:, hp, :])
        nc.gpsimd.collective_compute(
            "AllGather", mybir.AluOpType.bypass, replica_groups=GROUPS,
            ins=[cc_in[:]], outs=[cc_out[:]])

        with ExitStack() as eph:
            otp = eph.enter_context(tc.tile_pool(name="otp", bufs=2))
            ysb = eph.enter_context(tc.tile_pool(name="ysb", bufs=3))
            epsum = eph.enter_context(tc.tile_pool(name="epsum", bufs=4,
                                                   space="PSUM"))
            for sq in range(4):
                ot = otp.tile([128, 8, 512], F32R, tag="ot")
                nc.sync.dma_start(
                    ot[:],
                    cc_out.rearrange("(c p) s -> p c s", p=128)
                    [:, :, sq * 512:(sq + 1) * 512])
                for mc in range(4):
                    ep = epsum.tile([128, 512], F32, tag="ep")
                    for hc in range(8):
                        nc.tensor.matmul(
                            ep[:, 0:HD],
                            ot[:, hc, mc * 128:(mc + 1) * 128],
                            wo_t[:, hc, :],
                            start=(hc == 0), stop=(hc == 7))
                    yt = ysb.tile([128, HD], F32, tag="y")
                    nc.vector.tensor_copy(yt[:], ep[:, 0:HD])
                    nc.sync.dma_start(
                        y_d[(sq * 4 + mc) * 128:(sq * 4 + mc + 1) * 128, :],
                        yt[:])

    nc.compile()
    return nc


def _get_nc():
    global _cached
    if _cached is None:
        _cached = build()
    return _cached


def make_in_maps(x, w_qkv, w_out):
    x = np.asarray(x, dtype=np.float32)
    w_qkv = np.asarray(w_qkv, dtype=np.float32)
    w_out = np.asarray(w_out, dtype=np.float32)
    in_maps = []
    for c in range(N_CORES):
        b, r = c // 4, c % 4
        hs = r * HD                      # first qkv column of this core's heads
        in_maps.append({
            "x": np.ascontiguousarray(x[b]),
            "wq": np.ascontiguousarray(w_qkv[:, hs:hs + HD]),
            "wk": np.ascontiguousarray(w_qkv[:, E + hs:E + hs + HD]),
            "wv": np.ascontiguousarray(w_qkv[:, 2 * E + hs:2 * E + hs + HD]),
            "wo": np.ascontiguousarray(w_out[:, r * HD:(r + 1) * HD]),
        })
    return in_maps


def assemble(results):
    y = np.empty((B, S, E), dtype=np.float32)
    for c in range(N_CORES):
        b, r = c // 4, c % 4
        y[b, :, r * HD:(r + 1) * HD] = results[c]["y"]
    return y


def kernel(x, w_qkv, w_out):
    nc = _get_nc()
    res = run_bass_kernel_spmd(nc, make_in_maps(x, w_qkv, w_out),
                               list(range(N_CORES)))
    return assemble(res.results)


# revision 4
# speedup vs baseline: 5436.8968x; 5436.8968x over previous
"""Multi-head attention (B=2, S=2048, E=1024, H=16, D=64) on 8 Trainium2 cores.

Sharding: data-parallel over batch (2 groups of 4 cores), tensor-parallel over
heads within each group (4 heads per core, Megatron-style column-split qkv);
out_proj sharded over its output columns (each core owns 256 E-columns; the
w_out column slice arrives pre-sliced from the host so the SPMD program needs
no core-id logic).

Per-core pipeline (projection/logit matmuls in float32r = full PE rate with
~1e-4 rounding; the probability-weighted PV stage in bf16 because f32r
matmuls cannot target partial-partition PSUM destinations):
  x^T (PE transpose, exact fp32) -> q^T/k^T (lhsT=W, rhs=x^T) and v natural
  (lhsT=x^T, rhs=Wv) -> S^T = k q^T per head (row-packed K=64 head pairs)
  -> exp on ScalarE (scale=1/8 folded; no max subtraction: logits ~N(0,1),
  exp never overflows and matches the reference softmax mathematically)
  -> PV^T and all-ones-row denominators (col-packed head pairs, accumulated
  over the 16 S_k chunks in PSUM) -> reciprocal + multiply normalization
  -> AllGather(out^T) over the 4-core group -> out_proj column slice.
"""

import numpy as np
from contextlib import ExitStack

import concourse.tile as tile
from concourse import bacc, mybir
from concourse.bass_utils import run_bass_kernel_spmd
from concourse.masks import make_identity

B, S, E, H, D = 2, 2048, 1024, 16, 64
N_CORES = 8
HPC = 4            # heads per core
HD = HPC * D       # 256
GROUPS = [[0, 1, 2, 3], [4, 5, 6, 7]]

F32 = mybir.dt.float32
F32R = mybir.dt.float32r
BF16 = mybir.dt.bfloat16

_cached = None


def build(reps=1):
    nc = bacc.Bacc("TRN2", target_bir_lowering=False, debug=False,
                   num_devices=N_CORES)

    x_d = nc.dram_tensor("x", [S, E], F32, kind="ExternalInput").ap()
    wq_d = nc.dram_tensor("wq", [E, HD], F32R, kind="ExternalInput").ap()
    wk_d = nc.dram_tensor("wk", [E, HD], F32R, kind="ExternalInput").ap()
    wv_d = nc.dram_tensor("wv", [E, HD], F32R, kind="ExternalInput").ap()
    wo_d = nc.dram_tensor("wo", [E, HD], F32R, kind="ExternalInput").ap()
    y_d = nc.dram_tensor("y", [S, HD], F32, kind="ExternalOutput").ap()
    cc_in = nc.dram_tensor("cc_in", [2 * 128, S], F32R).ap()
    cc_out = nc.dram_tensor("cc_out", [8 * 128, S], F32R).ap()

    with tile.TileContext(nc) as tc, ExitStack() as ctx:
        glob = ctx.enter_context(tc.tile_pool(name="glob", bufs=1))
        qT_t = glob.tile([128, 2, S], F32R, tag="qT")     # q^T by head pair
        kT_t = glob.tile([128, 2, S], F32R, tag="kT")
        v_t = glob.tile([128, 16, HD], BF16, tag="v")     # v natural
        wo_t = glob.tile([128, 8, HD], F32R, tag="wo")
        outT_t = glob.tile([128, 2, S], F32R, tag="outT")
        ident = glob.tile([128, 128], F32, tag="ident")
        ones_f = glob.tile([128, D], F32, tag="ones_f")
        ones_t = glob.tile([128, D], BF16, tag="ones")

        make_identity(nc, ident[:])
        nc.gpsimd.memset(ones_f[:], 1.0)
        nc.vector.tensor_copy(ones_t[:], ones_f[:])
        nc.sync.dma_start(wo_t[:], wo_d.rearrange("(c p) n -> p c n", p=128))

        for _rep in range(reps):
            _emit_body(nc, tc, x_d, wq_d, wk_d, wv_d, y_d, cc_in, cc_out,
                       qT_t, kT_t, v_t, wo_t, outT_t, ident, ones_t)

    nc.compile()
    return nc


def _emit_body(nc, tc, x_d, wq_d, wk_d, wv_d, y_d, cc_in, cc_out,
               qT_t, kT_t, v_t, wo_t, outT_t, ident, ones_t):
    # ---- Phase A: x^T via PE transpose; Phase B: projections ----
    with ExitStack() as ab:
        abp = ab.enter_context(tc.tile_pool(name="abp", bufs=1))
        xload = ab.enter_context(tc.tile_pool(name="xload", bufs=3))
        tpsum = ab.enter_context(tc.tile_pool(name="tpsum", bufs=2,
                                              space="PSUM"))
        ppsum = ab.enter_context(tc.tile_pool(name="ppsum", bufs=2,
                                              space="PSUM"))
        xT_t = abp.tile([128, 8, S], F32R, tag="xT")
        wq_t = abp.tile([128, 8, HD], F32R, tag="wq")
        wk_t = abp.tile([128, 8, HD], F32R, tag="wk")
        wv_t = abp.tile([128, 8, HD], F32R, tag="wv")
        nc.sync.dma_start(wq_t[:], wq_d.rearrange("(c p) n -> p c n", p=128))
        nc.sync.dma_start(wk_t[:], wk_d.rearrange("(c p) n -> p c n", p=128))
        nc.sync.dma_start(wv_t[:], wv_d.rearrange("(c p) n -> p c n", p=128))

        for sc in range(16):
            xt = xload.tile([128, E], F32, tag="x")
            nc.sync.dma_start(xt[:], x_d[sc * 128:(sc + 1) * 128, :])
            for eq in range(2):
                tp = tpsum.tile([128, 512], F32, tag="tp")
                for i in range(4):
                    ec = eq * 4 + i
                    nc.tensor.transpose(
                        tp[:, i * 128:(i + 1) * 128],
                        xt[:, ec * 128:(ec + 1) * 128], ident[:])
                nc.vector.tensor_copy(
                    xT_t[:, eq * 4:(eq + 1) * 4, sc * 128:(sc + 1) * 128],
                    tp[:].rearrange("p (c n) -> p c n", c=4))

        for w_t, dst in ((wq_t, qT_t), (wk_t, kT_t)):
            for mc in range(2):
                for sq in range(4):
                    pp = ppsum.tile([128, 512], F32, tag="pp")
                    for ec in range(8):
                        nc.tensor.matmul(
                            pp[:],
                            w_t[:, ec, mc * 128:(mc + 1) * 128],
                            xT_t[:, ec, sq * 512:(sq + 1) * 512],
                            start=(ec == 0), stop=(ec == 7))
                    nc.scalar.copy(
                        dst[:, mc, sq * 512:(sq + 1) * 512], pp[:])
        for sc in range(16):
            pp = ppsum.tile([128, 512], F32, tag="pp")
            for ec in range(8):
                nc.tensor.matmul(
                    pp[:, 0:HD],
                    xT_t[:, ec, sc * 128:(sc + 1) * 128],
                    wv_t[:, ec, :],
                    start=(ec == 0), stop=(ec == 7))
            nc.scalar.copy(v_t[:, sc, :], pp[:, 0:HD])

    # ---- Phase C: attention in transposed space ----
    with ExitStack() as cph:
        stp = cph.enter_context(tc.tile_pool(name="stp", bufs=2,
                                             space="PSUM"))
        pvp = cph.enter_context(tc.tile_pool(name="pvp", bufs=2,
                                             space="PSUM"))
        dnp = cph.enter_context(tc.tile_pool(name="dnp", bufs=2,
                                             space="PSUM"))
        expp = cph.enter_context(tc.tile_pool(name="expp", bufs=3))
        recp = cph.enter_context(tc.tile_pool(name="recp", bufs=2))

        for qc in range(4):
            for hp in range(2):
                pv = pvp.tile([128, 512], F32, tag="pv")
                dn = dnp.tile([128, 512], F32, tag="dn")
                for kc in range(16):
                    st = stp.tile([128, 1024], F32, tag="st")
                    for par in range(2):   # row-packed K=64 pair
                        lo, hi = par * 64, (par + 1) * 64
                        nc.tensor.matmul(
                            st[:, par * 512:(par + 1) * 512],
                            kT_t[lo:hi, hp, kc * 128:(kc + 1) * 128],
                            qT_t[lo:hi, hp, qc * 512:(qc + 1) * 512],
                            start=True, stop=True)
                    ex = expp.tile([128, 1024], BF16, tag="ex")
                    nc.scalar.activation(
                        ex[:], st[:],
                        mybir.ActivationFunctionType.Exp, scale=0.125)
                    for par in range(2):   # col-packed M=64 pairs
                        h = 2 * hp + par
                        lo, hi = par * 64, (par + 1) * 64
                        exs = ex[:, par * 512:(par + 1) * 512]
                        nc.tensor.matmul(
                            pv[lo:hi, :],
                            v_t[:, kc, h * 64:(h + 1) * 64], exs,
                            start=(kc == 0), stop=(kc == 15))
                        nc.tensor.matmul(
                            dn[lo:hi, :], ones_t[:], exs,
                            start=(kc == 0), stop=(kc == 15))
                rc = recp.tile([128, 512], F32, tag="rc")
                nc.vector.reciprocal(rc[:], dn[:])
                nc.vector.tensor_mul(
                    outT_t[:, hp, qc * 512:(qc + 1) * 512], pv[:], rc[:])

    # ---- gather + out_proj (sharded over E columns via host wo slice) ----
    for hp in range(2):
        nc.sync.dma_start(cc_in[hp * 128:(hp + 1) * 128, :], outT_t[:, hp, :])
    nc.gpsimd.collective_compute(
        "AllGather", mybir.AluOpType.bypass, replica_groups=GROUPS,
        ins=[cc_in[:]], outs=[cc_out[:]])

    with ExitStack() as eph:
        otp = eph.enter_context(tc.tile_pool(name="otp", bufs=2))
        ysb = eph.enter_context(tc.tile_pool(name="ysb", bufs=3))
        epsum = eph.enter_context(tc.tile_pool(name="epsum", bufs=4,
                                               space="PSUM"))
        for sq in range(4):
            ot = otp.tile([128, 8, 512], F32R, tag="ot")
            nc.sync.dma_start(
                ot[:],
                cc_out.rearrange("(c p) s -> p c s", p=128)
                [:, :, sq * 512:(sq + 1) * 512])
            for mc in range(4):
                ep = epsum.tile([128, HD], F32, tag="ep")
                for hc in range(8):
                    nc.tensor.matmul(
                        ep[:],
                        ot[:, hc, mc * 128:(mc + 1) * 128],
                        wo_t[:, hc, :],
                        start=(hc == 0), stop=(hc == 7))
                yt = ysb.tile([128, HD], F32, tag="y")
                nc.vector.tensor_copy(yt[:], ep[:])
                nc.sync.dma_start(
                    y_d[(sq * 4 + mc) * 128:(sq * 4 + mc + 1) * 128, :],
                    yt[:])


def _get_nc():
    global _cached
    if _cached is None:
        _cached = build()
    return _cached


def make_in_maps(x, w_qkv, w_out):
    x = np.asarray(x, dtype=np.float32)
    w_qkv = np.asarray(w_qkv, dtype=np.float32)
    w_out = np.asarray(w_out, dtype=np.float32)
    in_maps = []
    for c in range(N_CORES):
        b, r = c // 4, c % 4
        hs = r * HD                      # first qkv column of this core's heads
        in_maps.append({
            "x": np.ascontiguousarray(x[b]),
            "wq": np.ascontiguousarray(w_qkv[:, hs:hs + HD]),
            "wk": np.ascontiguousarray(w_qkv[:, E + hs:E + hs + HD]),
            "wv": np.ascontiguousarray(w_qkv[:, 2 * E + hs:2 * E + hs + HD]),
            "wo": np.ascontiguousarray(w_out[:, r * HD:(r + 1) * HD]),
        })
    return in_maps


def assemble(results):
    y = np.empty((B, S, E), dtype=np.float32)
    for c in range(N_CORES):
        b, r = c // 4, c % 4
        y[b, :, r * HD:(r + 1) * HD] = results[c]["y"]
    return y


def kernel(x, w_qkv, w_out):
    nc = _get_nc()
    res = run_bass_kernel_spmd(nc, make_in_maps(x, w_qkv, w_out),
                               list(range(N_CORES)))
    return assemble(res.results)
